# revision 1
# baseline (speedup 1.0000x reference)
"""Trainium2 Bass kernel for the CustomAttn module (causal attention + ALiBi).

Sharding: heads across 8 cores (4 heads/core).  W_attn is column-sharded into
per-head q/k/v blocks, W_proj row-sharded correspondingly; each core produces a
partial [S, E] output and the host sums the 8 partials (plus the bias folds).

Layout trick: everything is computed transposed (qk^T: [feat, S]; attn^T:
[feat, S]) so that no on-device transposes are needed anywhere:
  - qk^T tiles come from matmul(lhsT=W_slice, rhs=X^T)
  - V comes naturally from matmul(lhsT=X^T, rhs=Wv)
  - scores^T[c, r] from matmul(lhsT=k^T, rhs=q^T), softmax'd unnormalized via
    exp (ACT) x Toeplitz mask tile (ALiBi decay * causal; only ~5 of 16
    key-blocks per query-block survive the fp32 underflow of the ALiBi decay)
  - attn^T from matmul(lhsT=[V | ones], rhs=probs^T): the ones column gives the
    softmax denominator as psum row 64 for free
  - the projection uses attn^T directly as lhsT.
The 1/sqrt(D) score scale is folded into Wq on the host.
"""

import sys
from contextlib import ExitStack

if "/opt/trn_rl_repo" not in sys.path:
    sys.path.insert(0, "/opt/trn_rl_repo")

import numpy as np

S = 2048
E = 2048
D = 64
HLOC = 4          # heads per core
N_CORES = 8
P = 128
NG = S // 512     # 4 column groups of 512
KE = E // 128     # 16 contraction tiles
ALIBI_M = 2.0 ** (-0.25)
NMASK = 5         # mask tiles for didx = kb - 4*qb + 1 in 0..4
BF16_ATTN = False  # bf16 probs/V path (2x DVE mask-mult, FWL on PV matmuls)


def _install_drain_patch():
    """This walrus build rejects a multi-wait SP Drain at the Tile kernel tail
    ("Too many sync wait commands"); split the waits into standalone
    EventSemaphore waits ahead of a bare drain."""
    from concourse import tile as _tile
    from concourse.vector_clock import ScopedClock

    if getattr(_tile.TileContext, "_drain_patch_installed", False):
        return

    def _patched(self, tick_clock, wait_clock):
        nc = self.nc
        probe = nc.sync.nop()
        wait_clock.add_sem_waits(
            probe.ins, ScopedClock({None: tick_clock.global_clock})
        )
        waits = list(probe.ins.sync_info.on_wait) if probe.ins.sync_info else []
        probe.ins.sync_info = None
        sems_by_name = {s.name: s for s in self.sems.allocated().values()}
        for w in waits:
            nc.sync.wait_ge(sems_by_name[w.ant_name], w.wait_value)
        nc.sync.drain()
        nc.all_engine_barrier()
        popped = nc._tile_sem_poison_stack.pop()
        assert popped is self._sem_poison
        nc.clear_and_free_semaphores(list(self.sems.allocated().values()))
        nc.all_engine_barrier()

    _tile.TileContext._drain_and_barrier = _patched
    _tile.TileContext._drain_patch_installed = True


MAX_WAITS = 1
MAX_WAITS_BY_OP = {"DMACopy": 1, "DMATranspose": 1, "Drain": 1, "NoOp": 1}


def _install_wait_split_patch():
    """Same walrus limitation, general form: instructions with more than
    MAX_WAITS sem-waits fail codegen ("Too many sync wait commands").  Hoist
    the excess waits onto standalone EventSemaphore instructions immediately
    before the instruction on the same engine queue (in-order execution makes
    that equivalent gating)."""
    from concourse import tile as _tile
    from concourse import mybir

    if getattr(_tile.TileContext, "_wait_split_installed", False):
        return
    orig_add = _tile.TileContext._add_instruction

    def _patched_add(self, inst):
        si = inst.sync_info
        lim = MAX_WAITS_BY_OP.get(type(inst).__name__.replace("Inst", ""), MAX_WAITS)
        try:
            opname = inst.concise_opcode()
        except Exception:
            opname = ""
        if opname in MAX_WAITS_BY_OP:
            lim = MAX_WAITS_BY_OP[opname]
        if si is not None and si.on_wait and len(si.on_wait) > lim:
            waits = list(si.on_wait)
            updates = list(si.on_update or [])
            excess = waits[lim:]
            for i in range(0, len(excess), MAX_WAITS):
                chunk = excess[i : i + MAX_WAITS]
                ev = mybir.InstEventSemaphore(
                    name=self.nc.get_next_instruction_name(),
                    engine=inst.engine,
                    ins=[],
                    outs=[],
                    sync_info=mybir.SyncInfo(on_wait=chunk, on_update=[]),
                )
                orig_add(self, ev)
            inst.sync_info = mybir.SyncInfo(on_wait=waits[:lim], on_update=updates)
        orig_add(self, inst)

    _tile.TileContext._add_instruction = _patched_add
    _tile.TileContext._wait_split_installed = True


def build_nc():
    import concourse.bass as bass
    import concourse.tile as tile
    from concourse import mybir

    _install_drain_patch()
    _install_wait_split_patch()

    F32 = mybir.dt.float32
    F32R = mybir.dt.float32r
    BF16 = mybir.dt.bfloat16
    PDT = BF16 if BF16_ATTN else F32R  # probs / V dtype
    ACTF = mybir.ActivationFunctionType
    MUL = mybir.AluOpType.mult

    nc = bass.Bass("TRN2", target_bir_lowering=False, debug=False)

    xt = nc.dram_tensor("xt", [E, S], F32R, kind="ExternalInput")
    wa = nc.dram_tensor("wa", [E, 768], F32R, kind="ExternalInput")
    bqk = nc.dram_tensor("bqk", [P, 4], F32, kind="ExternalInput")
    mt = nc.dram_tensor("mt", [P, NMASK * 512], mybir.dt.float32 if not BF16_ATTN else BF16, kind="ExternalInput")
    wp = nc.dram_tensor("wp", [256, E], F32R, kind="ExternalInput")
    out = nc.dram_tensor("out", [S, E], F32, kind="ExternalOutput")

    with tile.TileContext(nc) as tc, ExitStack() as ctx, nc.allow_low_precision(
        reason="bf16 probs path is deliberate; psum accumulation stays fp32"
    ):
        const = ctx.enter_context(tc.tile_pool(name="const", bufs=1))
        bqk_sb = const.tile([P, 4], F32)
        mt_sb = const.tile([P, NMASK * 512], PDT)
        wp_sb = const.tile([P, 2 * 2048], F32R)
        qk_sb = const.tile([P, 4 * 2048], F32R)
        v_sb = const.tile([P, 16 * 260], PDT)
        attn_sb = const.tile([P, 2 * 2048], F32R)
        ones_sb = const.tile([1, 64], F32)

        nc.vector.memset(ones_sb[:], 1.0)
        # fp32r memset fails the ISA check; fill the V ones-columns via a
        # DVE copy from an fp32 staging tile (DVE rounds on write).
        ones128_sb = const.tile([P, 64], F32)
        nc.vector.memset(ones128_sb[:], 1.0)
        ones128r_sb = const.tile([P, 64], F32R)
        nc.vector.tensor_copy(ones128r_sb[:], ones128_sb[:])
        v_ones_view = v_sb[:, :].rearrange("p (g c) -> p g c", c=65)[:, :, 64:65]
        nc.vector.tensor_copy(v_ones_view, ones128_sb[:, :, None])
        nc.gpsimd.dma_start(bqk_sb[:], bqk[:, :])
        nc.gpsimd.dma_start(mt_sb[:], mt[:, :])

        # ---- Phase A: qk^T [512, S] and V [S, 256] ----
        with ExitStack() as actx:
            wa_pool = actx.enter_context(tc.tile_pool(name="wap", bufs=1))
            wa_sb = wa_pool.tile([P, KE * 768], F32R)
            xt_pool = actx.enter_context(tc.tile_pool(name="xt", bufs=6))
            psA = actx.enter_context(tc.tile_pool(name="psA", bufs=1, space="PSUM"))
            for ng in range(NG):
                qk_ps = [
                    psA.tile([P, 512], F32, tag=f"qk{m}", name=f"qkps{m}") for m in range(4)
                ]
                v_ps = [
                    psA.tile([P, 256], F32, tag=f"v{j}", name=f"vps{j}") for j in range(4)
                ]
                for k in range(KE):
                    if ng == 0:
                        # split the 6MB preload across both DMA queues so the
                        # first xt tiles aren't stuck behind it
                        eng = nc.sync if k % 2 == 0 else nc.gpsimd
                        eng.dma_start(
                            wa_sb[:, k * 768 : (k + 1) * 768],
                            wa[k * P : (k + 1) * P, :],
                        )
                    xt_t = xt_pool.tile([P, 512], F32R)
                    nc.sync.dma_start(
                        xt_t[:], xt[k * P : (k + 1) * P, ng * 512 : (ng + 1) * 512]
                    )
                    xtr = xt_t[:]
                    for m in range(4):
                        nc.tensor.matmul(
                            qk_ps[m][:],
                            wa_sb[:, k * 768 + m * P : k * 768 + (m + 1) * P],
                            xtr,
                            start=(k == 0),
                            stop=(k == KE - 1),
                        )
                    for j in range(4):
                        nc.tensor.matmul(
                            v_ps[j][:],
                            xt_t[:, j * P : (j + 1) * P],
                            wa_sb[:, k * 768 + 512 : k * 768 + 768],
                            start=(k == 0),
                            stop=(k == KE - 1),
                        )
                for m in range(4):
                    nc.scalar.activation(
                        qk_sb[:, m * 2048 + ng * 512 : m * 2048 + (ng + 1) * 512],
                        qk_ps[m][:],
                        ACTF.Identity,
                        bias=bqk_sb[:, m : m + 1],
                    )
                for j in range(4):
                    blk = ng * 4 + j
                    dst = v_sb[:, blk * 260 : blk * 260 + 260].rearrange(
                        "p (h c) -> p h c", c=65
                    )[:, :, 0:64]
                    src = v_ps[j][:].rearrange("p (h c) -> p h c", c=64)
                    nc.vector.tensor_copy(dst, src)

        # ---- Phase B + C finely interleaved ----
        # Per qb: 4 head-chains, sc MMs two kb-steps ahead of pv MMs so the
        # in-order PE queue absorbs the exp->mask latency; projection (C)
        # MM-pairs for query group qb-1 (deps a full qb old) are spliced into
        # each step as PE filler, which also keeps the HAM clock warm.
        with ExitStack() as bctx:
            exp_pool = bctx.enter_context(tc.tile_pool(name="expp", bufs=8))
            p_pool = bctx.enter_context(tc.tile_pool(name="pp", bufs=20))
            rs_pool = bctx.enter_context(tc.tile_pool(name="rs", bufs=2))
            tmp_pool = bctx.enter_context(tc.tile_pool(name="tmpn", bufs=2))
            o_pool = bctx.enter_context(tc.tile_pool(name="op", bufs=6))
            ps_sc = bctx.enter_context(tc.tile_pool(name="pssc", bufs=3, space="PSUM"))
            ps_pv = bctx.enter_context(tc.tile_pool(name="pspv", bufs=1, space="PSUM"))
            ps_bc = bctx.enter_context(tc.tile_pool(name="psbc", bufs=1, space="PSUM"))
            for kk in range(2):
                nc.gpsimd.dma_start(
                    wp_sb[:, kk * 2048 : (kk + 1) * 2048],
                    wp[kk * P : (kk + 1) * P, :],
                )
            heads = (0, 1, 2, 3)

            def emit_c_pair(mb, n):
                o_ps = ps_sc.tile([P, 512], F32, tag="sc", name="o_ps")
                for kk in range(2):
                    nc.tensor.matmul(
                        o_ps[:],
                        attn_sb[:, kk * 2048 + mb * P : kk * 2048 + (mb + 1) * P],
                        wp_sb[:, kk * 2048 + n * 512 : kk * 2048 + (n + 1) * 512],
                        start=(kk == 0),
                        stop=(kk == 1),
                    )
                o_t = o_pool.tile([P, 512], F32)
                if (mb + n) % 2 == 0:
                    nc.scalar.activation(o_t[:], o_ps[:], ACTF.Copy)
                else:
                    nc.vector.tensor_copy(o_t[:], o_ps[:])
                nc.gpsimd.dma_start(
                    out[mb * P : (mb + 1) * P, n * 512 : (n + 1) * 512], o_t[:]
                )

            c_queue = []
            for qb in range(4):
                # per-qb reciprocal batch: rowsums of all 4 heads reshaped to
                # [128, 16] via DMA so the DVE reciprocal runs 16 elems/lane
                # instead of 512 elems on one lane.
                rsq_t = rs_pool.tile([P, 16], F32, tag="rsq", name="rsq_t", bufs=2)
                invq_t = rs_pool.tile([P, 16], F32R, tag="invq", name="invq_t", bufs=2)
                inv_sb = rs_pool.tile([1, 4 * 512], F32R, tag="invsb", name="inv_sb", bufs=2)
                kb_lo = max(0, 4 * qb - 1)
                kb_hi = 4 * qb + 3
                # ALiBi decay zeroes all but the first NCOLS[j] query-columns
                # of mask tile j; order the accumulation so the first matmul
                # (start=True) covers the full 512 columns.
                NCOLS = {0: 128, 1: 256, 2: 384, 3: 512, 4: 512}
                kbs = [
                    4 * qb + j - 1
                    for j in (3, 4, 2, 1, 0)
                    if kb_lo <= 4 * qb + j - 1 <= kb_hi
                ]
                pv_ps = {
                    h: ps_pv.tile([65, 512], F32, tag=f"pv{h}", name=f"pvps{h}")
                    for h in heads
                }
                pts = {}
                SKEW = 4
                nsteps = len(kbs) + SKEW
                # cap at 1/step: the remainder drains while the rowsum
                # reciprocal DMA round-trip is in flight (pre-bc-MM gap)
                fill = 1 if c_queue else 0
                for i in range(nsteps):
                    if i < len(kbs):
                        kb = kbs[i]
                        didx = kb - 4 * qb + 1  # 0..4
                        nc_q = NCOLS[didx]
                        for h in heads:
                            hb = (h % 2) * 64
                            q_ap = qk_sb[
                                hb : hb + 64,
                                (h // 2) * 2048 + qb * 512 : (h // 2) * 2048 + qb * 512 + nc_q,
                            ]
                            k_ap = qk_sb[
                                hb : hb + 64,
                                (2 + h // 2) * 2048 + kb * P : (2 + h // 2) * 2048 + (kb + 1) * P,
                            ]
                            sc_ps = ps_sc.tile([P, 512], F32, tag="sc")
                            nc.tensor.matmul(
                                sc_ps[:, :nc_q], k_ap, q_ap, start=True, stop=True
                            )
                            e_t = exp_pool.tile([P, 512], PDT)
                            nc.scalar.activation(
                                e_t[:, :nc_q], sc_ps[:, :nc_q], ACTF.Exp
                            )
                            p_t = p_pool.tile([P, 512], PDT)
                            nc.vector.tensor_tensor(
                                p_t[:, :nc_q],
                                e_t[:, :nc_q],
                                mt_sb[:, didx * 512 : didx * 512 + nc_q],
                                MUL,
                            )
                            pts[(kb, h)] = p_t
                    if i >= SKEW:
                        pkb = kbs[i - SKEW]
                        pnc = NCOLS[pkb - 4 * qb + 1]
                        for h in heads:
                            nc.tensor.matmul(
                                pv_ps[h][:, :pnc],
                                v_sb[:, pkb * 260 + 65 * h : pkb * 260 + 65 * h + 65],
                                pts.pop((pkb, h))[:, :pnc],
                                start=(pkb == kbs[0]),
                                stop=(pkb == kbs[-1]),
                            )
                    for _ in range(fill):
                        if c_queue:
                            emit_c_pair(*c_queue.pop(0))
                # rowsum -> reciprocal chain; drain leftover C pairs while the
                # DMA/recip round-trip is in flight.
                un_tiles = {}
                for h in heads:
                    un_t = rs_pool.tile(
                        [65, 512], F32, tag=f"un{h}", name=f"un_t{h}", bufs=1
                    )
                    un_tiles[h] = un_t
                    nc.scalar.activation(un_t[:], pv_ps[h][:], ACTF.Copy)
                    nc.sync.dma_start(rsq_t[:, h * 4 : (h + 1) * 4], un_t[64:65, :])
                nc.vector.reciprocal(invq_t[:], rsq_t[:])
                for h in heads:
                    nc.sync.dma_start(
                        inv_sb[0:1, h * 512 : (h + 1) * 512],
                        invq_t[:, h * 4 : (h + 1) * 4],
                    )
                while c_queue:
                    emit_c_pair(*c_queue.pop(0))
                for h in heads:
                    un_t = un_tiles[h]
                    bc_ps = ps_bc.tile([64, 512], F32, tag="bc")
                    nc.tensor.matmul(
                        bc_ps[:],
                        ones128r_sb[0:1, :],
                        inv_sb[0:1, h * 512 : (h + 1) * 512],
                        start=True,
                        stop=True,
                    )
                    col0 = (h // 2) * 2048 + qb * 512
                    if h % 2 == 0:
                        nc.vector.tensor_tensor(
                            attn_sb[0:64, col0 : col0 + 512],
                            un_t[0:64, :],
                            bc_ps[:],
                            MUL,
                        )
                    else:
                        # DVE lanes can't shift partitions; normalize at
                        # base 0 then DMA-shift to partitions 64..127.
                        tmp_t = tmp_pool.tile([64, 512], F32R)
                        nc.vector.tensor_tensor(
                            tmp_t[:], un_t[0:64, :], bc_ps[:], MUL
                        )
                        nc.sync.dma_start(
                            attn_sb[64:128, col0 : col0 + 512], tmp_t[:]
                        )
                c_queue.extend(
                    (mb, n) for mb in range(4 * qb, 4 * qb + 4) for n in range(4)
                )
            while c_queue:
                emit_c_pair(*c_queue.pop(0))

    return nc


def build_mask_tiles() -> np.ndarray:
    """mt[p, j*512 + r] = causal/ALiBi multiplicative mask for
    delta0 = (j - 1) * 128, i.e. key c = delta0 + 512*qb... relative offsets:
    t = c - r (tile-local: delta0 + ci - rj); keep exp(m*t) for t <= 0."""
    ci = np.arange(P)[:, None]
    rj = np.arange(512)[None, :]
    cols = []
    for j in range(NMASK):
        d0 = (j - 1) * 128
        t = d0 + ci - rj
        cols.append(np.where(t <= 0, np.exp(ALIBI_M * t), 0.0))
    arr = np.ascontiguousarray(np.concatenate(cols, axis=1))
    if BF16_ATTN:
        import ml_dtypes
        return arr.astype(ml_dtypes.bfloat16)
    return arr.astype(np.float32)


def make_in_maps(hidden_states, W_attn, b_attn, W_proj):
    x = np.asarray(hidden_states, dtype=np.float32).reshape(S, E)
    xt = np.ascontiguousarray(x.T)
    Wa = np.asarray(W_attn, dtype=np.float32)
    ba = np.asarray(b_attn, dtype=np.float32)
    Wp = np.asarray(W_proj, dtype=np.float32)
    mt = build_mask_tiles()
    in_maps = []
    for c in range(N_CORES):
        lo, hi = 256 * c, 256 * (c + 1)
        wq = Wa[:, lo:hi] * 0.125
        wk = Wa[:, E + lo : E + hi]
        wv = Wa[:, 2 * E + lo : 2 * E + hi]
        wa_shard = np.ascontiguousarray(np.concatenate([wq, wk, wv], axis=1))
        bqk = np.concatenate([ba[lo:hi] * 0.125, ba[E + lo : E + hi]])
        bqk_mat = np.ascontiguousarray(bqk.reshape(4, P).T)
        wp_shard = np.ascontiguousarray(Wp[lo:hi, :])
        in_maps.append(
            {"xt": xt, "wa": wa_shard, "bqk": bqk_mat, "mt": mt, "wp": wp_shard}
        )
    return in_maps


_NC_CACHE = {}


def kernel(hidden_states, W_attn, b_attn, W_proj, b_proj):
    from concourse.bass_utils import run_bass_kernel_spmd

    if "nc" not in _NC_CACHE:
        _NC_CACHE["nc"] = build_nc()
    nc = _NC_CACHE["nc"]

    in_maps = make_in_maps(hidden_states, W_attn, b_attn, W_proj)
    res = run_bass_kernel_spmd(nc, in_maps, core_ids=list(range(N_CORES)))

    out = np.zeros((S, E), dtype=np.float32)
    for c in range(N_CORES):
        out += res.results[c]["out"]
    ba = np.asarray(b_attn, dtype=np.float32)
    bp = np.asarray(b_proj, dtype=np.float32)
    Wp = np.asarray(W_proj, dtype=np.float32)
    # v-bias passes through softmax linearly (rows sum to 1): fold on host.
    out += ba[2 * E :] @ Wp + bp
    return out.reshape(1, S, E).astype(np.float32)



# revision 2
# speedup vs baseline: 1.0919x; 1.0919x over previous
"""Trainium2 Bass kernel for the CustomAttn module (causal attention + ALiBi).

Sharding: heads across 8 cores (4 heads/core).  W_attn is column-sharded into
per-head q/k/v blocks, W_proj row-sharded correspondingly; each core produces a
partial [S, E] output and the host sums the 8 partials (plus the bias folds).

Layout trick: everything is computed transposed (qk^T: [feat, S]; attn^T:
[feat, S]) so that no on-device transposes are needed anywhere:
  - qk^T tiles come from matmul(lhsT=W_slice, rhs=X^T)
  - V comes naturally from matmul(lhsT=X^T, rhs=Wv)
  - scores^T[c, r] from matmul(lhsT=k^T, rhs=q^T), softmax'd unnormalized via
    exp (ACT) x Toeplitz mask tile (ALiBi decay * causal; only ~5 of 16
    key-blocks per query-block survive the fp32 underflow of the ALiBi decay)
  - attn^T from matmul(lhsT=[V | ones], rhs=probs^T): the ones column gives the
    softmax denominator as psum row 64 for free
  - the projection uses attn^T directly as lhsT.
The 1/sqrt(D) score scale is folded into Wq on the host.

All tensors are bf16 on the wire and in SBUF (psum accumulation fp32);
the host sums the 8 bf16 partials in fp32.
"""

import sys
from contextlib import ExitStack

if "/opt/trn_rl_repo" not in sys.path:
    sys.path.insert(0, "/opt/trn_rl_repo")

import numpy as np

S = 2048
E = 2048
D = 64
HLOC = 4          # heads per core
N_CORES = 8
P = 128
NG = S // 512     # 4 column groups of 512
KE = E // 128     # 16 contraction tiles
ALIBI_M = 2.0 ** (-0.25)
NMASK = 5         # mask tiles for didx = kb - 4*qb + 1 in 0..4


def _install_drain_patch():
    """This walrus build rejects a multi-wait SP Drain at the Tile kernel tail
    ("Too many sync wait commands"); split the waits into standalone
    EventSemaphore waits ahead of a bare drain."""
    from concourse import tile as _tile
    from concourse.vector_clock import ScopedClock

    if getattr(_tile.TileContext, "_drain_patch_installed", False):
        return

    def _patched(self, tick_clock, wait_clock):
        nc = self.nc
        probe = nc.sync.nop()
        wait_clock.add_sem_waits(
            probe.ins, ScopedClock({None: tick_clock.global_clock})
        )
        waits = list(probe.ins.sync_info.on_wait) if probe.ins.sync_info else []
        probe.ins.sync_info = None
        sems_by_name = {s.name: s for s in self.sems.allocated().values()}
        for w in waits:
            nc.sync.wait_ge(sems_by_name[w.ant_name], w.wait_value)
        nc.sync.drain()
        nc.all_engine_barrier()
        popped = nc._tile_sem_poison_stack.pop()
        assert popped is self._sem_poison
        nc.clear_and_free_semaphores(list(self.sems.allocated().values()))
        nc.all_engine_barrier()

    _tile.TileContext._drain_and_barrier = _patched
    _tile.TileContext._drain_patch_installed = True


MAX_WAITS = 1
MAX_WAITS_BY_OP = {"DMACopy": 1, "DMATranspose": 1, "Drain": 1, "NoOp": 1}


def _install_wait_split_patch():
    """Same walrus limitation, general form: instructions with more than
    MAX_WAITS sem-waits fail codegen ("Too many sync wait commands").  Hoist
    the excess waits onto standalone EventSemaphore instructions immediately
    before the instruction on the same engine queue (in-order execution makes
    that equivalent gating)."""
    from concourse import tile as _tile
    from concourse import mybir

    if getattr(_tile.TileContext, "_wait_split_installed", False):
        return
    orig_add = _tile.TileContext._add_instruction

    def _patched_add(self, inst):
        si = inst.sync_info
        lim = MAX_WAITS_BY_OP.get(type(inst).__name__.replace("Inst", ""), MAX_WAITS)
        try:
            opname = inst.concise_opcode()
        except Exception:
            opname = ""
        if opname in MAX_WAITS_BY_OP:
            lim = MAX_WAITS_BY_OP[opname]
        if si is not None and si.on_wait and len(si.on_wait) > lim:
            waits = list(si.on_wait)
            updates = list(si.on_update or [])
            excess = waits[lim:]
            for i in range(0, len(excess), MAX_WAITS):
                chunk = excess[i : i + MAX_WAITS]
                ev = mybir.InstEventSemaphore(
                    name=self.nc.get_next_instruction_name(),
                    engine=inst.engine,
                    ins=[],
                    outs=[],
                    sync_info=mybir.SyncInfo(on_wait=chunk, on_update=[]),
                )
                orig_add(self, ev)
            inst.sync_info = mybir.SyncInfo(on_wait=waits[:lim], on_update=updates)
        orig_add(self, inst)

    _tile.TileContext._add_instruction = _patched_add
    _tile.TileContext._wait_split_installed = True


def build_nc():
    import concourse.bass as bass
    import concourse.tile as tile
    from concourse import mybir

    _install_drain_patch()
    _install_wait_split_patch()

    F32 = mybir.dt.float32
    F32R = mybir.dt.float32r
    BF16 = mybir.dt.bfloat16
    ACTF = mybir.ActivationFunctionType
    MUL = mybir.AluOpType.mult

    nc = bass.Bass("TRN2", target_bir_lowering=False, debug=False)

    xt = nc.dram_tensor("xt", [E, S], BF16, kind="ExternalInput")
    wa = nc.dram_tensor("wa", [E, 768], BF16, kind="ExternalInput")
    bqk = nc.dram_tensor("bqk", [P, 4], F32, kind="ExternalInput")
    mt = nc.dram_tensor("mt", [P, NMASK * 512], BF16, kind="ExternalInput")
    wp = nc.dram_tensor("wp", [256, E], BF16, kind="ExternalInput")
    out = nc.dram_tensor("out", [S, E], BF16, kind="ExternalOutput")

    with tile.TileContext(nc) as tc, ExitStack() as ctx, nc.allow_low_precision(
        reason="bf16 end-to-end is deliberate; psum accumulation stays fp32"
    ):
        const = ctx.enter_context(tc.tile_pool(name="const", bufs=1))
        bqk_sb = const.tile([P, 4], F32)
        mt_sb = const.tile([P, NMASK * 512], BF16)
        wp_sb = const.tile([P, 2 * 2048], BF16)
        qk_sb = const.tile([P, 4 * 2048], BF16)
        v_sb = const.tile([P, 16 * 260], BF16)
        attn_sb = const.tile([P, 2 * 2048], BF16)

        # ones column of V (per-head col 64) gives the softmax denominator
        ones128_sb = const.tile([P, 64], F32)
        nc.vector.memset(ones128_sb[:], 1.0)
        ones128r_sb = const.tile([P, 64], F32R)
        nc.vector.tensor_copy(ones128r_sb[:], ones128_sb[:])
        v_ones_view = v_sb[:, :].rearrange("p (g c) -> p g c", c=65)[:, :, 64:65]
        nc.vector.tensor_copy(v_ones_view, ones128_sb[:, :, None])
        nc.gpsimd.dma_start(bqk_sb[:], bqk[:, :])

        # ---- Phase A: qk^T [512, S] and V [S, 256] ----
        with ExitStack() as actx:
            wa_pool = actx.enter_context(tc.tile_pool(name="wap", bufs=1))
            wa_t = [
                wa_pool.tile([P, 768], BF16, tag=f"wa{k}", name=f"wa{k}")
                for k in range(KE)
            ]
            xt_pool = actx.enter_context(tc.tile_pool(name="xt", bufs=6))
            psA = actx.enter_context(tc.tile_pool(name="psA", bufs=1, space="PSUM"))
            for ng in range(NG):
                qk_ps = [
                    psA.tile([P, 512], F32, tag=f"qk{m}", name=f"qkps{m}") for m in range(4)
                ]
                v_ps = [
                    psA.tile([P, 256], F32, tag=f"v{j}", name=f"vps{j}") for j in range(4)
                ]
                for k in range(KE):
                    if ng == 0:
                        # per-k wa tiles: the first matmul only waits on
                        # wa_t[0] + xt tile 0 instead of the full preload
                        eng = nc.sync if k % 2 == 0 else nc.gpsimd
                        eng.dma_start(wa_t[k][:], wa[k * P : (k + 1) * P, :])
                    xt_t = xt_pool.tile([P, 512], BF16)
                    nc.sync.dma_start(
                        xt_t[:], xt[k * P : (k + 1) * P, ng * 512 : (ng + 1) * 512]
                    )
                    for m in range(4):
                        nc.tensor.matmul(
                            qk_ps[m][:],
                            wa_t[k][:, m * P : (m + 1) * P],
                            xt_t[:],
                            start=(k == 0),
                            stop=(k == KE - 1),
                        )
                    for j in range(4):
                        nc.tensor.matmul(
                            v_ps[j][:],
                            xt_t[:, j * P : (j + 1) * P],
                            wa_t[k][:, 512:768],
                            start=(k == 0),
                            stop=(k == KE - 1),
                        )
                for m in range(4):
                    nc.scalar.activation(
                        qk_sb[:, m * 2048 + ng * 512 : m * 2048 + (ng + 1) * 512],
                        qk_ps[m][:],
                        ACTF.Identity,
                        bias=bqk_sb[:, m : m + 1],
                    )
                for j in range(4):
                    blk = ng * 4 + j
                    dst = v_sb[:, blk * 260 : blk * 260 + 260].rearrange(
                        "p (h c) -> p h c", c=65
                    )[:, :, 0:64]
                    src = v_ps[j][:].rearrange("p (h c) -> p h c", c=64)
                    nc.vector.tensor_copy(dst, src)
            # constants needed later; emitted after phase A DMAs so they don't
            # delay the first matmuls
            nc.gpsimd.dma_start(mt_sb[:], mt[:, :])

        # ---- Phase B + C finely interleaved ----
        # Per qb: 4 head-chains, sc MMs two kb-steps ahead of pv MMs so the
        # in-order PE queue absorbs the exp->mask latency; projection (C)
        # MM-pairs for query group qb-1 (deps a full qb old) are spliced into
        # each step as PE filler, which also keeps the HAM clock warm.
        with ExitStack() as bctx:
            exp_pool = bctx.enter_context(tc.tile_pool(name="expp", bufs=8))
            p_pool = bctx.enter_context(tc.tile_pool(name="pp", bufs=20))
            rs_pool = bctx.enter_context(tc.tile_pool(name="rs", bufs=2))
            tmp_pool = bctx.enter_context(tc.tile_pool(name="tmpn", bufs=2))
            o_pool = bctx.enter_context(tc.tile_pool(name="op", bufs=6))
            ps_sc = bctx.enter_context(tc.tile_pool(name="pssc", bufs=3, space="PSUM"))
            ps_pv = bctx.enter_context(tc.tile_pool(name="pspv", bufs=1, space="PSUM"))
            ps_bc = bctx.enter_context(tc.tile_pool(name="psbc", bufs=1, space="PSUM"))
            for kk in range(2):
                nc.gpsimd.dma_start(
                    wp_sb[:, kk * 2048 : (kk + 1) * 2048],
                    wp[kk * P : (kk + 1) * P, :],
                )
            heads = (0, 1, 2, 3)

            def emit_c_pair(mb, n):
                o_ps = ps_sc.tile([P, 512], F32, tag="sc", name="o_ps")
                for kk in range(2):
                    nc.tensor.matmul(
                        o_ps[:],
                        attn_sb[:, kk * 2048 + mb * P : kk * 2048 + (mb + 1) * P],
                        wp_sb[:, kk * 2048 + n * 512 : kk * 2048 + (n + 1) * 512],
                        start=(kk == 0),
                        stop=(kk == 1),
                    )
                o_t = o_pool.tile([P, 512], BF16)
                if (mb + n) % 2 == 0:
                    nc.scalar.activation(o_t[:], o_ps[:], ACTF.Copy)
                else:
                    nc.vector.tensor_copy(o_t[:], o_ps[:])
                nc.gpsimd.dma_start(
                    out[mb * P : (mb + 1) * P, n * 512 : (n + 1) * 512], o_t[:]
                )

            c_queue = []
            for qb in range(4):
                inv_sb = rs_pool.tile([1, 4 * 512], F32R, tag="invsb", name="inv_sb", bufs=2)
                kb_lo = max(0, 4 * qb - 1)
                kb_hi = 4 * qb + 3
                # ALiBi decay zeroes all but the first NCOLS[j] query-columns
                # of mask tile j; order the accumulation so the first matmul
                # (start=True) covers the full 512 columns.
                NCOLS = {0: 128, 1: 256, 2: 384, 3: 512, 4: 512}
                kbs = [
                    4 * qb + j - 1
                    for j in (3, 4, 2, 1, 0)
                    if kb_lo <= 4 * qb + j - 1 <= kb_hi
                ]
                pv_ps = {
                    h: ps_pv.tile([65, 512], F32, tag=f"pv{h}", name=f"pvps{h}")
                    for h in heads
                }
                pts = {}
                SKEW = 4
                nsteps = len(kbs) + SKEW
                # cap at 1/step: the remainder drains during the per-qb
                # normalization chain (pre-bc-MM gap)
                fill = 1 if c_queue else 0
                for i in range(nsteps):
                    if i < len(kbs):
                        kb = kbs[i]
                        didx = kb - 4 * qb + 1  # 0..4
                        nc_q = NCOLS[didx]
                        for h in heads:
                            hb = (h % 2) * 64
                            q_ap = qk_sb[
                                hb : hb + 64,
                                (h // 2) * 2048 + qb * 512 : (h // 2) * 2048 + qb * 512 + nc_q,
                            ]
                            k_ap = qk_sb[
                                hb : hb + 64,
                                (2 + h // 2) * 2048 + kb * P : (2 + h // 2) * 2048 + (kb + 1) * P,
                            ]
                            sc_ps = ps_sc.tile([P, 512], F32, tag="sc")
                            nc.tensor.matmul(
                                sc_ps[:, :nc_q], k_ap, q_ap, start=True, stop=True
                            )
                            e_t = exp_pool.tile([P, 512], BF16)
                            nc.scalar.activation(
                                e_t[:, :nc_q], sc_ps[:, :nc_q], ACTF.Exp
                            )
                            p_t = p_pool.tile([P, 512], BF16)
                            nc.vector.tensor_tensor(
                                p_t[:, :nc_q],
                                e_t[:, :nc_q],
                                mt_sb[:, didx * 512 : didx * 512 + nc_q],
                                MUL,
                            )
                            pts[(kb, h)] = p_t
                    if i >= SKEW:
                        pkb = kbs[i - SKEW]
                        pnc = NCOLS[pkb - 4 * qb + 1]
                        for h in heads:
                            nc.tensor.matmul(
                                pv_ps[h][:, :pnc],
                                v_sb[:, pkb * 260 + 65 * h : pkb * 260 + 65 * h + 65],
                                pts.pop((pkb, h))[:, :pnc],
                                start=(pkb == kbs[0]),
                                stop=(pkb == kbs[-1]),
                            )
                    for _ in range(fill):
                        if c_queue:
                            emit_c_pair(*c_queue.pop(0))
                # rowsum -> reciprocal -> broadcast-MM -> normalize chain;
                # drain leftover C pairs while it is in flight.
                un_tiles = {}
                for h in heads:
                    un_t = rs_pool.tile(
                        [65, 512], F32, tag=f"un{h}", name=f"un_t{h}", bufs=1
                    )
                    un_tiles[h] = un_t
                    nc.scalar.activation(un_t[:], pv_ps[h][:], ACTF.Copy)
                    nc.vector.reciprocal(
                        inv_sb[0:1, h * 512 : (h + 1) * 512], un_t[64:65, :]
                    )
                while c_queue:
                    emit_c_pair(*c_queue.pop(0))
                for h in heads:
                    un_t = un_tiles[h]
                    bc_ps = ps_bc.tile([64, 512], F32, tag="bc")
                    nc.tensor.matmul(
                        bc_ps[:],
                        ones128r_sb[0:1, :],
                        inv_sb[0:1, h * 512 : (h + 1) * 512],
                        start=True,
                        stop=True,
                    )
                    col0 = (h // 2) * 2048 + qb * 512
                    if h % 2 == 0:
                        nc.vector.tensor_tensor(
                            attn_sb[0:64, col0 : col0 + 512],
                            un_t[0:64, :],
                            bc_ps[:],
                            MUL,
                        )
                    else:
                        # DVE lanes can't shift partitions; normalize at
                        # base 0 then DMA-shift to partitions 64..127.
                        tmp_t = tmp_pool.tile([64, 512], BF16)
                        nc.vector.tensor_tensor(
                            tmp_t[:], un_t[0:64, :], bc_ps[:], MUL
                        )
                        nc.sync.dma_start(
                            attn_sb[64:128, col0 : col0 + 512], tmp_t[:]
                        )
                c_queue.extend(
                    (mb, n) for mb in range(4 * qb, 4 * qb + 4) for n in range(4)
                )
            while c_queue:
                emit_c_pair(*c_queue.pop(0))

    return nc


def build_mask_tiles() -> np.ndarray:
    """mt[p, j*512 + r] = causal/ALiBi multiplicative mask for
    delta0 = (j - 1) * 128, i.e. key c = delta0 + 512*qb... relative offsets:
    t = c - r (tile-local: delta0 + ci - rj); keep exp(m*t) for t <= 0."""
    import ml_dtypes

    ci = np.arange(P)[:, None]
    rj = np.arange(512)[None, :]
    cols = []
    for j in range(NMASK):
        d0 = (j - 1) * 128
        t = d0 + ci - rj
        cols.append(np.where(t <= 0, np.exp(ALIBI_M * t), 0.0))
    arr = np.ascontiguousarray(np.concatenate(cols, axis=1))
    return arr.astype(ml_dtypes.bfloat16)


def make_in_maps(hidden_states, W_attn, b_attn, W_proj):
    import ml_dtypes

    BF = ml_dtypes.bfloat16
    x = np.asarray(hidden_states, dtype=np.float32).reshape(S, E)
    xt = np.ascontiguousarray(x.T).astype(BF)
    Wa = np.asarray(W_attn, dtype=np.float32)
    ba = np.asarray(b_attn, dtype=np.float32)
    Wp = np.asarray(W_proj, dtype=np.float32)
    mt = build_mask_tiles()
    in_maps = []
    for c in range(N_CORES):
        lo, hi = 256 * c, 256 * (c + 1)
        wq = Wa[:, lo:hi] * 0.125
        wk = Wa[:, E + lo : E + hi]
        wv = Wa[:, 2 * E + lo : 2 * E + hi]
        wa_shard = np.ascontiguousarray(
            np.concatenate([wq, wk, wv], axis=1)
        ).astype(BF)
        bqk = np.concatenate([ba[lo:hi] * 0.125, ba[E + lo : E + hi]])
        bqk_mat = np.ascontiguousarray(bqk.reshape(4, P).T)
        wp_shard = np.ascontiguousarray(Wp[lo:hi, :]).astype(BF)
        in_maps.append(
            {"xt": xt, "wa": wa_shard, "bqk": bqk_mat, "mt": mt, "wp": wp_shard}
        )
    return in_maps


_NC_CACHE = {}


def kernel(hidden_states, W_attn, b_attn, W_proj, b_proj):
    from concourse.bass_utils import run_bass_kernel_spmd

    if "nc" not in _NC_CACHE:
        _NC_CACHE["nc"] = build_nc()
    nc = _NC_CACHE["nc"]

    in_maps = make_in_maps(hidden_states, W_attn, b_attn, W_proj)
    res = run_bass_kernel_spmd(nc, in_maps, core_ids=list(range(N_CORES)))

    out = np.zeros((S, E), dtype=np.float32)
    for c in range(N_CORES):
        out += np.asarray(res.results[c]["out"], dtype=np.float32)
    ba = np.asarray(b_attn, dtype=np.float32)
    bp = np.asarray(b_proj, dtype=np.float32)
    Wp = np.asarray(W_proj, dtype=np.float32)
    # v-bias passes through softmax linearly (rows sum to 1): fold on host.
    out += ba[2 * E :] @ Wp + bp
    return out.reshape(1, S, E).astype(np.float32)
